# revision 21
# baseline (speedup 1.0000x reference)
"""Trainium2 Bass kernel for nn_AttExplainer (segment_reduce).

Data-parallel over motifs M across 8 NeuronCores:
 - each core gathers its 12500 motifs' node embeddings with indirect DMAs
   (one [P,1]-offset gather per neighbor slot k, 128 rows each),
 - mean over K=16 neighbors -> embed shard, logits = embed @ att / 10,
   exp(logits) locally (logits are in [-1.5,1.5], so softmax needs no max),
 - one-hot segment matmuls accumulate seg_sum(embed|ones) and
   seg_sum(embed*exp) into PSUM over all chunks,
 - AllReduce #1 carries {seg_SC, seg_E, gsum}; then a = exp/gsum,
   add_b = seg_E/gsum, add_inv_b = seg_S - add_b,
 - pass 2 computes seg_sum(embed * sigmoid(1000a-100)), AllReduce #2,
   mean_pool = that / max(counts,1),
 - host concatenates a/embed shards; pools read from core 0.

Self-contained: all shapes hardcoded; needs numpy + concourse (axon PJRT).
"""
import numpy as np

import concourse.bass as bass
import concourse.bass_isa as bass_isa
import concourse.bacc as bacc
import concourse.mybir as mybir
import concourse.tile as tile
from concourse import bass_utils
from concourse.masks import make_identity

# Problem constants
N_NODES = 200000
N_MOTIFS = 100000
K = 16
D = 128
G = 256            # num_graphs
NCORES = 8
P = 128
MC = N_MOTIFS // NCORES          # 12500 motifs per core
NCHUNK = (MC + P - 1) // P       # 98
MCP = NCHUNK * P                 # 12544 padded
E1 = D + 1                       # embed row + ones column (counts)
PAD_BATCH = 999.0                # pad motifs match no graph

_CACHE = {}

import os
DBG_NCORES = int(os.environ.get("KERNEL_NCORES", NCORES))
DBG_NO_CC = os.environ.get("KERNEL_NO_CC", "") == "1"


def _build():
    if "nc" in _CACHE:
        return _CACHE["nc"]

    nc = bacc.Bacc("TRN2", target_bir_lowering=False, debug=False,
                   enable_asserts=True, num_devices=DBG_NCORES)

    f32 = mybir.dt.float32
    i32 = mybir.dt.int32

    n_emb_t = nc.dram_tensor("n_emb", [N_NODES, D], f32, kind="ExternalInput")
    midx_t = nc.dram_tensor("midx", [P, NCHUNK * K], i32, kind="ExternalInput")
    batchf_t = nc.dram_tensor("batchf", [P, NCHUNK], f32, kind="ExternalInput")
    padmask_t = nc.dram_tensor("padmask", [P, 1], f32, kind="ExternalInput")
    iota_t = nc.dram_tensor("iota", [P, G], f32, kind="ExternalInput")
    att_t = nc.dram_tensor("att", [P, D], f32, kind="ExternalInput")

    a_out = nc.dram_tensor("a_out", [MCP], f32, kind="ExternalOutput")
    embed_out = nc.dram_tensor("embed_out", [MCP, D], f32, kind="ExternalOutput")
    meanp_out = nc.dram_tensor("meanp_out", [G, D], f32, kind="ExternalOutput")
    addb_out = nc.dram_tensor("addb_out", [G, D], f32, kind="ExternalOutput")
    addinvb_out = nc.dram_tensor("addinvb_out", [G, D], f32, kind="ExternalOutput")

    RG = [list(range(DBG_NCORES))]
    AX = mybir.AxisListType.X
    OP = mybir.AluOpType

    with tile.TileContext(nc) as tc:
        with tc.tile_pool(name="const", bufs=1) as cp, \
             tc.tile_pool(name="gath", bufs=10) as gp, \
             tc.tile_pool(name="work", bufs=4) as wp, \
             tc.tile_pool(name="psum", bufs=1, space="PSUM") as pp, \
             tc.tile_pool(name="dram", bufs=1, space="DRAM") as dp:

            # ---- persistent SBUF state ----
            idx_all = cp.tile([P, NCHUNK * K], i32)
            batch_all = cp.tile([P, NCHUNK], f32)
            padmask = cp.tile([P, 1], f32)
            iota = cp.tile([P, G], f32)
            att = cp.tile([P, D], f32)
            embed_all = cp.tile([P, NCHUNK * E1], f32)
            logit = cp.tile([P, NCHUNK], f32)
            expt = cp.tile([P, NCHUNK], f32)
            ident = cp.tile([P, P], f32)

            nc.sync.dma_start(out=idx_all[:], in_=midx_t[:, :])
            nc.sync.dma_start(out=batch_all[:], in_=batchf_t[:, :])
            nc.sync.dma_start(out=padmask[:], in_=padmask_t[:, :])
            nc.sync.dma_start(out=iota[:], in_=iota_t[:, :])
            nc.sync.dma_start(out=att[:], in_=att_t[:, :])
            make_identity(nc, ident[:])
            # ones column per chunk (for counts): embed_all[:, c*E1 + D] = 1
            emb3 = embed_all[:].rearrange("p (c e) -> p c e", c=NCHUNK, e=E1)
            nc.vector.memset(emb3[:, :, D:E1], 1.0)

            ps_SC_lo = pp.tile([P, E1], f32)
            ps_SC_hi = pp.tile([P, E1], f32)
            ps_E_lo = pp.tile([P, D], f32)
            ps_E_hi = pp.tile([P, D], f32)

            # ---- Phase A ----
            for c in range(NCHUNK):
                gath = gp.tile([P, K * D], f32, tag="gath")
                for k in range(K):
                    j = c * K + k
                    nc.gpsimd.indirect_dma_start(
                        out=gath[:, k * D:(k + 1) * D], out_offset=None,
                        in_=n_emb_t[:, :],
                        in_offset=bass.IndirectOffsetOnAxis(
                            ap=idx_all[:, j:j + 1], axis=0),
                    )
                emb = embed_all[:, c * E1:c * E1 + D]
                emb129 = embed_all[:, c * E1:c * E1 + E1]
                ssum = wp.tile([P, D], f32, tag="ssum")
                nc.vector.reduce_sum(
                    out=ssum[:],
                    in_=gath[:].rearrange("p (k d) -> p d k", k=K, d=D),
                    axis=AX)
                nc.vector.tensor_scalar_mul(emb, ssum[:], 1.0 / K)
                nc.sync.dma_start(out=embed_out[c * P:(c + 1) * P, :], in_=emb)

                scr = wp.tile([P, D], f32, tag="scr")
                nc.vector.tensor_tensor(out=scr[:], in0=emb, in1=att[:],
                                        op=OP.mult)
                nc.vector.reduce_sum(out=logit[:, c:c + 1], in_=scr[:], axis=AX)
                nc.scalar.activation(expt[:, c:c + 1], logit[:, c:c + 1],
                                     mybir.ActivationFunctionType.Exp, scale=0.1)
                if c == NCHUNK - 1 and MC % P != 0:
                    # zero the pad motifs' exp so they don't pollute gsum
                    nc.vector.tensor_tensor(
                        out=expt[:, c:c + 1], in0=expt[:, c:c + 1],
                        in1=padmask[:], op=OP.mult)

                eexp = wp.tile([P, D], f32, tag="eexp")
                nc.vector.tensor_scalar_mul(eexp[:], emb, expt[:, c:c + 1])

                oneh = wp.tile([P, G], f32, tag="oneh")
                nc.vector.tensor_tensor(
                    out=oneh[:], in0=batch_all[:, c:c + 1].to_broadcast([P, G]),
                    in1=iota[:], op=OP.is_equal)

                st = dict(start=(c == 0), stop=(c == NCHUNK - 1))
                nc.tensor.matmul(out=ps_SC_lo[:], lhsT=oneh[:, 0:128], rhs=emb129, **st)
                nc.tensor.matmul(out=ps_E_lo[:], lhsT=oneh[:, 0:128], rhs=eexp[:], **st)
                nc.tensor.matmul(out=ps_SC_hi[:], lhsT=oneh[:, 128:256], rhs=emb129, **st)
                nc.tensor.matmul(out=ps_E_hi[:], lhsT=oneh[:, 128:256], rhs=eexp[:], **st)

            # ---- local gsum + AllReduce #1 ----
            rowsum = cp.tile([P, 1], f32)
            nc.vector.reduce_sum(out=rowsum[:], in_=expt[:, :], axis=AX)
            gsum_col = cp.tile([P, 1], f32)
            nc.gpsimd.partition_all_reduce(
                gsum_col[:], rowsum[:], channels=P,
                reduce_op=bass_isa.ReduceOp.add)

            # pack: 0:129 SC_lo | 129:258 SC_hi | 258:386 E_lo | 386:514 E_hi | 514 gsum
            poolsA = cp.tile([P, 515], f32)
            nc.vector.tensor_copy(poolsA[:, 0:129], ps_SC_lo[:])
            nc.vector.tensor_copy(poolsA[:, 129:258], ps_SC_hi[:])
            nc.vector.tensor_copy(poolsA[:, 258:386], ps_E_lo[:])
            nc.vector.tensor_copy(poolsA[:, 386:514], ps_E_hi[:])
            nc.vector.tensor_copy(poolsA[:, 514:515], gsum_col[:])

            cc1_in = dp.tile([P, 515], mybir.dt.float32)
            cc1_out = dp.tile([P, 515], mybir.dt.float32, addr_space="Shared")
            nc.sync.dma_start(out=cc1_in[:], in_=poolsA[:])
            if DBG_NO_CC:
                nc.sync.dma_start(out=cc1_out[:], in_=cc1_in[:])
            else:
                nc.gpsimd.collective_compute(
                    "AllReduce", OP.add, replica_groups=RG,
                    ins=[cc1_in.opt()], outs=[cc1_out.opt()])
            poolsB = cp.tile([P, 515], f32)
            nc.sync.dma_start(out=poolsB[:], in_=cc1_out[:])

            # ---- epilogue: a, add_b, add_inv_b ----
            rgsum = cp.tile([P, 1], f32)
            nc.vector.reciprocal(rgsum[:], poolsB[:, 514:515])

            addb_lo = cp.tile([P, D], f32)
            addb_hi = cp.tile([P, D], f32)
            nc.vector.tensor_scalar_mul(addb_lo[:], poolsB[:, 258:386], rgsum[:])
            nc.vector.tensor_scalar_mul(addb_hi[:], poolsB[:, 386:514], rgsum[:])
            nc.sync.dma_start(out=addb_out[0:128, :], in_=addb_lo[:])
            nc.sync.dma_start(out=addb_out[128:256, :], in_=addb_hi[:])

            addinv_lo = cp.tile([P, D], f32)
            addinv_hi = cp.tile([P, D], f32)
            nc.vector.tensor_tensor(out=addinv_lo[:], in0=poolsB[:, 0:128],
                                    in1=addb_lo[:], op=OP.subtract)
            nc.vector.tensor_tensor(out=addinv_hi[:], in0=poolsB[:, 129:257],
                                    in1=addb_hi[:], op=OP.subtract)
            nc.sync.dma_start(out=addinvb_out[0:128, :], in_=addinv_lo[:])
            nc.sync.dma_start(out=addinvb_out[128:256, :], in_=addinv_hi[:])

            # a = exp * rgsum ; c = sigmoid(1000 a - 100)
            acol = cp.tile([P, NCHUNK], f32)
            nc.vector.tensor_scalar_mul(acol[:], expt[:, :], rgsum[:])
            ctile = cp.tile([P, NCHUNK], f32)
            bias_m100 = cp.tile([P, 1], f32)
            nc.vector.memset(bias_m100[:], -100.0)
            nc.scalar.activation(ctile[:], acol[:],
                                 mybir.ActivationFunctionType.Sigmoid,
                                 bias=bias_m100[:], scale=1000.0)

            # a out via PE transpose
            ps_t = pp.tile([P, P], f32)
            nc.tensor.transpose(out=ps_t[:NCHUNK, :], in_=acol[:, :],
                                identity=ident[:])
            a_t = cp.tile([P, P], f32)
            nc.vector.tensor_copy(a_t[:NCHUNK, :], ps_t[:NCHUNK, :])
            nc.sync.dma_start(
                out=a_out[:].rearrange("(c p) -> c p", c=NCHUNK, p=P),
                in_=a_t[:NCHUNK, :])

            # ---- Phase C: gated pool ----
            psC_lo = pp.tile([P, D], f32)
            psC_hi = pp.tile([P, D], f32)
            for c in range(NCHUNK):
                emb = embed_all[:, c * E1:c * E1 + D]
                newt = wp.tile([P, D], f32, tag="newt")
                nc.vector.tensor_scalar_mul(newt[:], emb, ctile[:, c:c + 1])
                oneh = wp.tile([P, G], f32, tag="oneh2")
                nc.vector.tensor_tensor(
                    out=oneh[:], in0=batch_all[:, c:c + 1].to_broadcast([P, G]),
                    in1=iota[:], op=OP.is_equal)
                st = dict(start=(c == 0), stop=(c == NCHUNK - 1))
                nc.tensor.matmul(out=psC_lo[:], lhsT=oneh[:, 0:128], rhs=newt[:], **st)
                nc.tensor.matmul(out=psC_hi[:], lhsT=oneh[:, 128:256], rhs=newt[:], **st)

            segC = cp.tile([P, 2 * D], f32)
            nc.vector.tensor_copy(segC[:, 0:D], psC_lo[:])
            nc.vector.tensor_copy(segC[:, D:2 * D], psC_hi[:])
            cc2_in = dp.tile([P, 2 * D], mybir.dt.float32)
            cc2_out = dp.tile([P, 2 * D], mybir.dt.float32, addr_space="Shared")
            nc.sync.dma_start(out=cc2_in[:], in_=segC[:])
            if DBG_NO_CC:
                nc.sync.dma_start(out=cc2_out[:], in_=cc2_in[:])
            else:
                nc.gpsimd.collective_compute(
                    "AllReduce", OP.add, replica_groups=RG,
                    ins=[cc2_in.opt()], outs=[cc2_out.opt()])
            segC2 = cp.tile([P, 2 * D], f32)
            nc.sync.dma_start(out=segC2[:], in_=cc2_out[:])

            # mean_pool = segC / max(counts, 1); counts at cols 128 / 257
            mcnt_lo = cp.tile([P, 1], f32)
            mcnt_hi = cp.tile([P, 1], f32)
            nc.vector.tensor_scalar_max(mcnt_lo[:], poolsB[:, 128:129], 1.0)
            nc.vector.tensor_scalar_max(mcnt_hi[:], poolsB[:, 257:258], 1.0)
            rc_lo = cp.tile([P, 1], f32)
            rc_hi = cp.tile([P, 1], f32)
            nc.vector.reciprocal(rc_lo[:], mcnt_lo[:])
            nc.vector.reciprocal(rc_hi[:], mcnt_hi[:])
            meanp_lo = cp.tile([P, D], f32)
            meanp_hi = cp.tile([P, D], f32)
            nc.vector.tensor_scalar_mul(meanp_lo[:], segC2[:, 0:D], rc_lo[:])
            nc.vector.tensor_scalar_mul(meanp_hi[:], segC2[:, D:2 * D], rc_hi[:])
            nc.sync.dma_start(out=meanp_out[0:128, :], in_=meanp_lo[:])
            nc.sync.dma_start(out=meanp_out[128:256, :], in_=meanp_hi[:])

    nc.compile()
    _CACHE["nc"] = nc
    return nc


def _make_in_maps(n_emb, att_embedding, motif_idx, batch):
    n_emb = np.ascontiguousarray(np.asarray(n_emb, dtype=np.float32))
    att = np.asarray(att_embedding, dtype=np.float32)
    midx = np.asarray(motif_idx, dtype=np.int32)
    batchf = np.asarray(batch, dtype=np.float32)

    iota = np.ascontiguousarray(
        np.broadcast_to(np.arange(G, dtype=np.float32), (P, G)))
    att_rep = np.ascontiguousarray(np.broadcast_to(att, (P, D)))

    in_maps = []
    for i in range(DBG_NCORES):
        sl = slice(i * MC, (i + 1) * MC)
        mi = np.zeros((MCP, K), dtype=np.int32)
        mi[:MC] = midx[sl]
        mi_t = np.ascontiguousarray(
            mi.reshape(NCHUNK, P, K).transpose(1, 0, 2).reshape(P, NCHUNK * K))
        bf = np.full((MCP,), PAD_BATCH, dtype=np.float32)
        bf[:MC] = batchf[sl]
        bf_t = np.ascontiguousarray(bf.reshape(NCHUNK, P).T)
        pm = np.ones((P, 1), dtype=np.float32)
        pm[MC % P:, 0] = 0.0
        in_maps.append({
            "n_emb": n_emb,
            "midx": mi_t,
            "batchf": bf_t,
            "padmask": pm,
            "iota": iota,
            "att": att_rep,
        })
    return in_maps


def _assemble(results):
    n = len(results)
    a = np.concatenate([results[i]["a_out"][:MC] for i in range(n)])
    embed = np.concatenate([results[i]["embed_out"][:MC] for i in range(n)])
    mean_pool = results[0]["meanp_out"]
    add_b = results[0]["addb_out"]
    add_inv_b = results[0]["addinvb_out"]
    return (np.asarray(a, dtype=np.float32), np.asarray(embed, dtype=np.float32),
            np.asarray(mean_pool, dtype=np.float32),
            np.asarray(add_b, dtype=np.float32),
            np.asarray(add_inv_b, dtype=np.float32))


def run(n_emb, att_embedding, motif_idx, batch, num_graphs=G, trace=False,
        **spmd_kwargs):
    nc = _build()
    in_maps = _make_in_maps(n_emb, att_embedding, motif_idx, batch)
    res = bass_utils.run_bass_kernel_spmd(
        nc, in_maps, core_ids=list(range(DBG_NCORES)), trace=trace, **spmd_kwargs)
    return _assemble(res.results), res


def kernel(n_emb, att_embedding, motif_idx, batch, num_graphs=G):
    out, _ = run(n_emb, att_embedding, motif_idx, batch, num_graphs)
    return out
